# revision 1
# baseline (speedup 1.0000x reference)
"""Distributed Trainium2 kernel for nn_Attention_7722351198977. (baseline restore)"""

import os as _os

import ml_dtypes
import numpy as np

import concourse.bass as bass
import concourse.bacc as bacc
import concourse.mybir as mybir
import concourse.tile as tile
from concourse.bass_utils import run_bass_kernel_spmd
from concourse.masks import make_identity

F32 = mybir.dt.float32
NO_F32R = _os.environ.get("ATTN_NO_F32R", "0") == "1"
F32R = mybir.dt.float32 if NO_F32R else mybir.dt.float32r
BF16 = mybir.dt.bfloat16
B, T, D, H = 8, 1024, 1024, 64
NC = 8
TC = T // NC
NPAIR = TC // 2
MASK_VAL = -1.0e9

MM_F32R = _os.environ.get("ATTN_MM_F32R", "0") == "1"
PHASE = int(_os.environ.get("ATTN_PHASE", "9"))


def _mm(ap):
    return ap


def build(num_cores: int = NC) -> bass.Bass:
    nc = bacc.Bacc(
        "TRN2", target_bir_lowering=False, debug=False, num_devices=num_cores
    )

    xT = nc.declare_dram_parameter("xT", [D, B * TC], F32R, isOutput=False)
    wqk = nc.declare_dram_parameter("wqk", [D, 2 * H], F32R, isOutput=False)
    wv = nc.declare_dram_parameter("wv", [D, H], F32R, isOutput=False)
    relp = nc.declare_dram_parameter("relp", [NPAIR, TC, T], BF16, isOutput=False)
    mask = nc.declare_dram_parameter("mask", [TC, T], F32, isOutput=False)
    out_e = nc.declare_dram_parameter("out", [B * TC, H], F32, isOutput=True)

    stage_ds = [
        nc.dram_tensor(f"stg{hf}", [8, 8, 8, T], BF16) for hf in range(2)
    ]
    cc_in = nc.dram_tensor("cc_in", [TC, B * TC], F32)
    cc_out = nc.dram_tensor("cc_out", [NC * TC, B * TC], F32, addr_space="Shared")

    Copy = mybir.ActivationFunctionType.Copy
    Exp = mybir.ActivationFunctionType.Exp

    with tile.TileContext(nc) as tc:
        with (
            tc.tile_pool(name="const", bufs=1) as constp,
            tc.tile_pool(name="big", bufs=1) as bigp,
            tc.tile_pool(name="relps", bufs=2) as relpp,
            tc.tile_pool(name="attn", bufs=3) as attnp,
            tc.tile_pool(name="small", bufs=8) as smallp,
            tc.tile_pool(name="psc", bufs=1, space="PSUM") as ps_sc,
            tc.tile_pool(name="pst", bufs=2, space="PSUM") as ps_t,
            tc.tile_pool(name="pso", bufs=2, space="PSUM") as ps_o,
            tc.tile_pool(name="pbias", bufs=1, space="PSUM") as ps_b,
        ):
            ident = constp.tile([128, 128], F32)
            make_identity(nc, ident[:])
            ident_bf = constp.tile([128, 128], BF16)
            nc.vector.tensor_copy(ident_bf[:], ident[:])
            zero_sb = constp.tile([128, 512], F32)
            nc.gpsimd.memset(zero_sb[:], 0.0)

            wqk_sb = constp.tile([128, 8, 2 * H], F32R)
            nc.sync.dma_start(
                out=wqk_sb[:], in_=wqk.rearrange("(c p) m -> p c m", p=128)
            )
            wv_sb = constp.tile([128, 8, H], F32R)
            nc.sync.dma_start(
                out=wv_sb[:], in_=wv.rearrange("(c p) m -> p c m", p=128)
            )
            mask_sb = constp.tile([128, T], F32)
            nc.sync.dma_start(out=mask_sb[:], in_=mask[:])

            psB = [
                ps_b.tile([128, 512], F32, tag=f"psB{h}", name=f"psB{h}")
                for h in range(2)
            ]

            with tc.tile_pool(name="xtp", bufs=1) as xtp:
                xT_sb = xtp.tile([128, 8, B * TC], F32R)
                xTv = xT.rearrange("(c p) r -> p c r", p=128)
                nc.sync.dma_start(out=xT_sb[:, 0:4, :], in_=xTv[:, 0:4, :])
                nc.scalar.dma_start(out=xT_sb[:, 4:8, :], in_=xTv[:, 4:8, :])

                for h2 in range(2):
                    for c in range(8):
                        nc.tensor.matmul(
                            psB[h2][:],
                            _mm(wqk_sb[:, c, :]),
                            _mm(xT_sb[:, c, h2 * 512 : (h2 + 1) * 512]),
                            start=(c == 0),
                            stop=(c == 7),
                        )
                qT_sb = constp.tile([H, B * TC], F32R)
                kT_loc = constp.tile([H, B * TC], F32R)
                for h2 in range(2):
                    sl = slice(h2 * 512, (h2 + 1) * 512)
                    nc.scalar.activation(qT_sb[:, sl], psB[h2][0:H, :], Copy)
                    nc.scalar.activation(
                        kT_loc[:, sl], psB[h2][H:128, :], Copy, scale=8.0
                    )

                for b in range(B):
                    for c in range(8):
                        nc.tensor.matmul(
                            psB[0][:, b * H : (b + 1) * H],
                            _mm(xT_sb[:, c, b * TC : (b + 1) * TC]),
                            _mm(wv_sb[:, c, :]),
                            start=(c == 0),
                            stop=(c == 7),
                        )
                v_loc = constp.tile([128, B * H], F32)
                nc.scalar.activation(v_loc[:], psB[0][:], Copy)

            qstage = constp.tile([128, NPAIR * 16], BF16)
            nc.gpsimd.memset(qstage[:], 0.0)
            qsrc = qT_sb.bitcast(F32).rearrange("c (b pp s) -> c pp s b", b=B, pp=NPAIR, s=2)
            qdst_lo = qstage[0:64, :].rearrange(
                "c (pp s b) -> c pp s b", pp=NPAIR, s=2, b=B
            )
            qdst_hi = qstage[64:128, :].rearrange(
                "c (pp s b) -> c pp s b", pp=NPAIR, s=2, b=B
            )
            nc.vector.tensor_copy(qdst_lo[:, :, 0, :], qsrc[:, :, 0, :])
            nc.vector.tensor_copy(qdst_hi[:, :, 1, :], qsrc[:, :, 1, :])

            if PHASE >= 2:
                nc.sync.dma_start(out=cc_in[0:H, :], in_=kT_loc[:].bitcast(F32))
                nc.sync.dma_start(
                    out=cc_in[H:TC, :].rearrange("p (a c) -> (p a) c", a=2),
                    in_=v_loc[:],
                )
                nc.gpsimd.collective_compute(
                    "AllGather",
                    mybir.AluOpType.bypass,
                    replica_groups=[list(range(num_cores))],
                    ins=[cc_in[:]],
                    outs=[cc_out[:]],
                )
                kT_nat = bigp.tile([H, B, NC, TC], F32R)
                nc.gpsimd.dma_start(
                    out=kT_nat[:],
                    in_=cc_out.rearrange(
                        "(j tw p) (b t) -> tw p b j t", j=NC, tw=2, p=H, b=B
                    )[0].bitcast(F32R),
                )
                # v as [tl, (j, b, h)]: 2KB contiguous runs per (tl, j)
                v_nat = bigp.tile([128, NC, B, H], BF16)
                nc.gpsimd.dma_start(
                    out=v_nat[:],
                    in_=cc_out.rearrange(
                        "(j tw th) (tp b h) -> tw (th tp) j b h",
                        j=NC, tw=2, th=64, tp=2, b=B, h=H,
                    )[1],
                )


            for h2 in range(2):
                nc.scalar.activation(psB[h2][:], zero_sb[:], Copy)

            bias_tc = bigp.tile([128, B * T], BF16, tag="btc", name="bias_tc")
            bias_sb = bigp.tile([128, 8, T], BF16)
            for half in range(2 if PHASE >= 3 else 0):
                for q in range(32):
                    p = half * 32 + q
                    quad, pm4 = q // 4, q % 4
                    if p % 8 == 0:
                        relp_ch = relpp.tile([TC, 8, T], BF16, tag="rp")
                        nc.sync.dma_start(
                            out=relp_ch[:],
                            in_=relp.rearrange(
                                "(ch pr) p t -> ch p pr t", ch=8
                            )[p // 8],
                        )
                    relp_t = relp_ch[:, p % 8, :]
                    for h2 in range(2):
                        nc.tensor.matmul(
                            psB[h2][32 * pm4 : 32 * pm4 + 16, :],
                            _mm(qstage[:, 16 * p : 16 * p + 16]),
                            _mm(relp_t[:, h2 * 512 : (h2 + 1) * 512]),
                            tile_position=(0, 32 * pm4),
                            start=True,
                            stop=True,
                        )
                    if pm4 == 3:
                        for h2 in range(2):
                            sl = slice(h2 * 512, (h2 + 1) * 512)
                            nc.scalar.activation(
                                bias_sb[:, quad, sl], psB[h2][:], Copy
                            )
                stg = stage_ds[half].rearrange(
                    "q (pm tw) b v -> pm (tw b) q v", pm=4, tw=2
                )
                for pm in range(4):
                    nc.sync.dma_start(
                        out=stg[pm],
                        in_=bias_sb[32 * pm : 32 * pm + 16, :, :],
                    )
                r0h = 64 * half
                nc.sync.dma_start(
                    out=bias_tc[r0h : r0h + 64, :].rearrange(
                        "q (b v) -> q b v", b=B
                    ),
                    in_=stage_ds[half].rearrange("q pt b v -> (q pt) b v"),
                )
                for b in range(B):
                    nc.vector.tensor_tensor(
                        out=bias_tc[r0h : r0h + 64, b * T : (b + 1) * T],
                        in0=bias_tc[r0h : r0h + 64, b * T : (b + 1) * T],
                        in1=mask_sb[r0h : r0h + 64, :],
                        op=mybir.AluOpType.add,
                    )

            for b in range(B if PHASE >= 4 else 0):
                psS0 = ps_sc.tile([128, 512], F32, tag="sc0", name="psS0")
                psS1 = ps_sc.tile([128, 512], F32, tag="sc1", name="psS1")
                psS = [psS0, psS1]
                lhs_q = qT_sb[:, b * TC : (b + 1) * TC]
                for h2 in range(2):
                    nc.tensor.matmul(
                        psS[h2][:],
                        _mm(lhs_q),
                        _mm(kT_nat[:, b, 4 * h2 : 4 * h2 + 4, :]),
                        start=True,
                        stop=True,
                    )
                attn_pre = attnp.tile([128, T], F32, tag="apre")
                for h2 in range(2):
                    sl = slice(h2 * 512, (h2 + 1) * 512)
                    nc.vector.tensor_tensor(
                        out=attn_pre[:, sl],
                        in0=psS[h2][:],
                        in1=bias_tc[:, b * T + h2 * 512 : b * T + (h2 + 1) * 512],
                        op=mybir.AluOpType.add,
                    )
                if PHASE < 5:
                    continue
                negmax = smallp.tile([128, 1], F32, tag="nmax")
                nc.vector.reduce_max(
                    negmax[:], attn_pre[:], axis=mybir.AxisListType.X,
                    negate=True,
                )
                attn_e = attnp.tile([128, T], BF16, tag="aexp")
                denom = smallp.tile([128, 1], F32, tag="den")
                nc.scalar.activation(
                    attn_e[:],
                    attn_pre[:],
                    Exp,
                    bias=negmax[:],
                    scale=1.0,
                    accum_out=denom[:],
                )
                if PHASE < 6:
                    continue
                attnT = attnp.tile([128, T], BF16, tag="aT")
                for g in range(2):
                    psT = ps_t.tile([128, 512], BF16, tag="pt")
                    for s4 in range(4):
                        s8 = 4 * g + s4
                        nc.tensor.transpose(
                            psT[:, 128 * s4 : 128 * s4 + 128],
                            attn_e[:, 128 * s8 : 128 * s8 + 128],
                            ident_bf[:],
                        )
                    nc.scalar.activation(
                        attnT[:, 512 * g : 512 * g + 512], psT[:], Copy
                    )
                if PHASE < 7:
                    continue
                psO = ps_o.tile([128, H], F32, tag="po")
                for s8 in range(8):
                    nc.tensor.matmul(
                        psO[:],
                        _mm(attnT[:, 128 * s8 : 128 * s8 + 128]),
                        _mm(v_nat[:, s8, b, :]),
                        start=(s8 == 0),
                        stop=(s8 == 7),
                    )
                rden = smallp.tile([128, 1], F32, tag="rden")
                nc.vector.reciprocal(rden[:], denom[:])
                out_sb = smallp.tile([128, H], F32, tag="osb")
                nc.scalar.activation(out_sb[:], psO[:], Copy, scale=rden[:])
                nc.sync.dma_start(
                    out=out_e[b * TC : (b + 1) * TC, :], in_=out_sb[:]
                )
            if PHASE < 9:
                dummy = smallp.tile([64, H], F32, tag="osb")
                nc.vector.tensor_copy(dummy[:], qT_sb[:, 0:H])
                for r in range(0, B * TC, 64):
                    nc.sync.dma_start(out=out_e[r : r + 64, :], in_=dummy[:])
    nc.compile()
    return nc


_CACHE: dict = {}


def _get_nc():
    if "nc" not in _CACHE:
        _CACHE["nc"] = build(NC)
    return _CACHE["nc"]


def _prep_inputs(x, Wq, Wk, Wv, relpos):
    x = np.ascontiguousarray(x, dtype=np.float32)
    relpos = np.ascontiguousarray(relpos, dtype=np.float32)
    wqk = np.ascontiguousarray(
        np.concatenate([Wq, Wk], axis=1), dtype=np.float32
    )
    wv = np.ascontiguousarray(Wv, dtype=np.float32)
    in_maps = []
    for i in range(NC):
        xs = x[:, TC * i : TC * (i + 1), :]
        xT = np.ascontiguousarray(
            xs.transpose(2, 0, 1).reshape(D, B * TC)
        )
        rp = relpos[TC * i : TC * (i + 1)]
        relp = np.ascontiguousarray(
            rp.transpose(0, 2, 1).reshape(NPAIR, TC, T)
        ).astype(ml_dtypes.bfloat16)
        tl = np.arange(TC)[:, None]
        vv = np.arange(T)[None, :]
        msk = np.where(vv <= TC * i + tl, 0.0, MASK_VAL).astype(np.float32)
        in_maps.append(
            {"xT": xT, "wqk": wqk, "wv": wv, "relp": relp, "mask": msk}
        )
    return in_maps


def run_sharded(in_maps, trace=False, **kw):
    nc = _get_nc()
    return run_bass_kernel_spmd(
        nc, in_maps, core_ids=list(range(NC)), trace=trace, **kw
    )


def kernel(x, Wq, Wk, Wv, relpos):
    in_maps = _prep_inputs(x, Wq, Wk, Wv, relpos)
    res = run_sharded(in_maps, trace=False)
    out = np.empty((B, T, H), dtype=np.float32)
    for i in range(NC):
        out[:, TC * i : TC * (i + 1), :] = (
            res.results[i]["out"].reshape(B, TC, H)
        )
    return out

